# revision 38
# baseline (speedup 1.0000x reference)
"""AngularPenaltySMLoss (CosFace) on 8 TRN2 NeuronCores.

Strategy: data-parallel over the batch N=4096. Each core owns 512 samples
and computes the FULL class dimension C=100000 for them, so no collective
is needed: each core emits its partial sum of (log den_i - s*tgt_i) and
the host sums the 8 partials (the mean + margin fold is host-side too).

Per core, per (n-tile i, chunk of 4 c-tiles):
  - logits [128 n x <=2048 c] = fp8 DoubleRow matmuls, xT stationary,
    W^T moving, K=512 contracted as 2 accumulating 256-row steps into a
    4-bank PSUM group.
  - consumer split so neither engine exceeds the ~2.07us of PE work per
    group: ScalarE takes banks 0-2 (exact Exp, per-partition scale
    a[n] = S/||x_n||, fused row-sum accumulator); VectorE takes bank 3
    via the Schraudolph fast-exp
    bit trick + row reduce. No zero padding: the last c-tile is 160 wide.
  - norms via DVE only (squares fused mul+reduce, rsqrt via quake bit
    trick + 2 Newton steps) so ScalarE never loads the Sqrt table set.
  - target logits from host-gathered W[labels] rows: one fused DVE
    mul+reduce per n-tile, interleaved mid-loop.
  - epilogue log via the inverse-Schraudolph bit trick on DVE (no Ln
    table load); the only ACT table set ever loaded is Exp's.

W^T is cast to fp8 once on the host and shared by all 8 cores (full C).
"""

import ml_dtypes
import numpy as np

from concourse import bacc, mybir, tile
from concourse.bass_utils import run_bass_kernel_spmd

N, D, C = 4096, 512, 100000
N_CORES = 8
NS = N // N_CORES               # 512 samples per core
S = 30.0
SM = 10.5                       # S * margin(0.35)
CT = 512                        # c-tile width (one PSUM bank of f32)
NCH = (C + 4 * CT - 1) // (4 * CT)   # 49 chunks of up to 4 banks

# Schraudolph fast-exp constants (DVE offload): exp(x) ~= bitcast_f32(
# int32(x * 2^23/ln2 + (127*2^23 - C))), C=486411 zeroes the mean error
EXP_A = float(2 ** 23 / np.log(2))
EXP_B = float(1065353216 - 486411)
# inverse (fast-log): ln(x) ~= (bitcast_i32(x) - B) * ln2/2^23,
# B = 2^23*(127 - 0.0430357) zeroes the mean error
LOG_K = float(np.log(2) / 2 ** 23)
LOG_B = 1065353216.0 - round(2 ** 23 * 0.0430357)
RSQ_MAGIC = 1597463007.0        # 0x5f3759df quake rsqrt seed

f32 = mybir.dt.float32
bf16 = mybir.dt.bfloat16
fp8 = mybir.dt.float8e4
i32 = mybir.dt.int32
np_bf16 = ml_dtypes.bfloat16
np_fp8 = mybir.dt.np(mybir.dt.float8e4)
AF = mybir.ActivationFunctionType
ALU = mybir.AluOpType
AX = mybir.AxisListType


def build(ns=NS, d=D, c=C, ct=CT, n_cores=N_CORES, act_w=1536, inplace=1,
          prefetch=3, use_ttr=0, use_quake=1, use_fastlog=1, rhs_sliced=1,
          split=1):
    # use_ttr=1 (InstTensorTensorReduce) crashes real HW (NRT INTERNAL)
    # even though CoreSim accepts it -- probed 2026-08-07; keep it off.
    ni = ns // 128                 # 4 n-tiles
    nk8 = d // 256                 # 2 DoubleRow K-steps
    nhb = (c + 4 * ct - 1) // (4 * ct)   # host 2048-wide row blocks
    # chunk descriptors (host_block, col_off, width), one per host block
    # (splitting the first block into single-bank chunks was tried and
    # regressed: tiny groups are consumer-limited, ~1.05us for 0.43us of
    # PE work)
    chunks = [(hb, 0, min(4 * ct, c - 4 * ct * hb)) for hb in range(nhb)]
    nch = len(chunks)

    nc = bacc.Bacc("TRN2", target_bir_lowering=False, debug=False,
                   num_devices=n_cores)
    x_nat = nc.dram_tensor("x_nat", [ns, d], bf16, kind="ExternalInput").ap()
    xtb_d = nc.dram_tensor("xtb", [d, ns], fp8, kind="ExternalInput").ap()
    wl = nc.dram_tensor("wl", [ns, d], bf16, kind="ExternalInput").ap()
    # W^T stored chunk-major ([nch*d, 4*ct], last chunk zero-padded) so
    # every DMA stride stays small (the flat [d, C] layout would need a
    # 100000-byte partition stride)
    wt = nc.dram_tensor("wt", [nhb * d, 4 * ct], fp8,
                        kind="ExternalInput").ap()
    out = nc.dram_tensor("out", [1, 1], f32, kind="ExternalOutput").ap()

    with tile.TileContext(nc) as tc:
        with (
            tc.tile_pool(name="persist", bufs=1) as pp,
            tc.tile_pool(name="stage", bufs=3) as sp,
            tc.tile_pool(name="wbuf",
                         bufs=(prefetch + 1) * (8 if rhs_sliced == 2 else 1)
                         ) as wbp,
            tc.tile_pool(name="scr", bufs=2) as scp,
        ):
            xtb = [pp.tile([128, 2, ns], fp8, tag=f"xtb{g}",
                           name=f"xtbs{g}") for g in range(nk8)]
            parts = pp.tile([128, ni * nch * 2], f32, tag="parts",
                            name="parts")
            ss = pp.tile([128, ni], f32, tag="ss", name="ss")
            tgt = pp.tile([128, ni], f32, tag="tgt", name="tgt")
            a_all = pp.tile([128, ni], f32, tag="a_all", name="a_all")
            a2_all = pp.tile([128, ni], f32, tag="a2_all", name="a2_all")
            ones = pp.tile([128, 1], f32, tag="ones", name="ones")

            # xT resident in SBUF -- gates the first matmuls
            for g in range(nk8):
                nc.sync.dma_start(
                    xtb[g][:],
                    xtb_d[g * 256:(g + 1) * 256, :].rearrange(
                        "(s p) n -> p s n", s=2))

            # W-chunk staging: one wide DMA per chunk, 4 DoubleRow k-pair
            # planes so rhs slices [:, 2g:2g+2, :] feed the matmuls
            def stage_chunk(ci, fine=False):
                hb, c0, cw = chunks[ci]
                rows = wt[hb * d:(hb + 1) * d, c0:c0 + cw]
                if rhs_sliced == 2:
                    # exact baseline staging: per-(g, jc) [128, 2, ct]
                    # tiles, rhs APs are whole tiles
                    wbt = {}
                    for jc in range((cw + ct - 1) // ct):
                        w0, w1 = jc * ct, min((jc + 1) * ct, cw)
                        for g in range(nk8):
                            wb = wbp.tile([128, 2, ct], fp8, tag="wbe",
                                          name="wbe")
                            nc.sync.dma_start(
                                wb[:, :, :w1 - w0],
                                rows[g * 256:(g + 1) * 256, w0:w1].rearrange(
                                    "(s p) c -> p s c", s=2))
                            wbt[(g, jc)] = wb
                    return wbt
                if not rhs_sliced:
                    # baseline-shaped staging: one [128, 2, cw] tile per
                    # DoubleRow k-group, rhs APs never slice the pair dim
                    wbg = []
                    for g in range(nk8):
                        wb = wbp.tile([128, 2, 4 * ct], fp8, tag=f"wbg{g}",
                                      name=f"wbg{g}")
                        nc.sync.dma_start(
                            wb[:, :, :cw],
                            rows[g * 256:(g + 1) * 256, :cw].rearrange(
                                "(s p) c -> p s c", s=2))
                        wbg.append(wb)
                    return wbg
                wb = wbp.tile([128, 4, 4 * ct], fp8, tag="wb", name="wb")
                if fine:   # per-bank DMAs so the first matmul starts ASAP
                    for jc in range((cw + ct - 1) // ct):
                        w0, w1 = jc * ct, min((jc + 1) * ct, cw)
                        nc.sync.dma_start(
                            wb[:, :, w0:w1],
                            rows[:, w0:w1].rearrange("(s p) c -> p s c", s=4))
                else:
                    nc.sync.dma_start(
                        wb[:, :, :cw],
                        rows[:, :cw].rearrange("(s p) c -> p s c", s=4))
                return wb

            # chunk0 first (gates the first matmuls), then the norm x
            # tiles (gate the first ACT at ~t+4us), then the deeper W
            # prefetch -- all squeezed into the same ~300GB/s DMA stream
            staged = {0: stage_chunk(0, fine=True)}

            # norms, DVE only: ss = sum(x^2)/S^2, a = rsqrt(ss), then the
            # Schraudolph pre-scale a2 = a * 2^23/ln2
            for i in range(ni):
                xa = sp.tile([128, d], bf16, tag="xa", name="xa")
                nc.sync.dma_start(xa[:], x_nat[i * 128:(i + 1) * 128, :])
                sq = scp.tile([128, d], f32, tag="sq", name="sq")
                if use_ttr:
                    nc.vector.tensor_tensor_reduce(
                        out=sq[:], in0=xa[:], in1=xa[:], scale=1.0 / (S * S),
                        scalar=0.0, op0=ALU.mult, op1=ALU.add,
                        accum_out=ss[:, i:i + 1])
                else:
                    nc.vector.tensor_mul(sq[:], xa[:], xa[:])
                    nc.vector.reduce_sum(ss[:, i:i + 1], sq[:], axis=AX.X)
            if not use_ttr:
                # fold the S factor: a = rsqrt(ss / S^2) = S / ||x||
                nc.vector.tensor_scalar_mul(ss[:], ss[:], 1.0 / (S * S))
            if use_quake:
                y0i = pp.tile([128, ni], i32, tag="y0i", name="y0i")
                yt = pp.tile([128, ni], f32, tag="yt", name="yt")
                rt = pp.tile([128, ni], f32, tag="rt", name="rt")
                # seed: bits(y0) = MAGIC - bits(ss)/2 (int arithmetic done
                # in f32; the low bits it rounds away are noise the Newton
                # steps absorb)
                nc.vector.tensor_scalar(out=y0i[:], in0=ss[:].bitcast(i32),
                                        scalar1=-0.5, scalar2=RSQ_MAGIC,
                                        op0=ALU.mult, op1=ALU.add)
                ycur = y0i[:].bitcast(f32)
                for it in range(2):
                    dst_y = a_all if it == 1 else yt
                    nc.vector.tensor_mul(rt[:], ycur, ycur)
                    nc.vector.tensor_mul(rt[:], rt[:], ss[:])
                    nc.vector.tensor_scalar(out=rt[:], in0=rt[:],
                                            scalar1=-0.5, scalar2=1.5,
                                            op0=ALU.mult, op1=ALU.add)
                    nc.vector.tensor_mul(dst_y[:], ycur, rt[:])
                    ycur = dst_y[:]
            else:
                ut = pp.tile([128, ni], f32, tag="ut", name="ut")
                nc.scalar.activation(ut[:], ss[:], AF.Sqrt)
                nc.vector.reciprocal(a_all[:], ut[:])
            nc.vector.tensor_scalar_mul(a2_all[:], a_all[:], EXP_A)
            nc.vector.memset(ones[:], 1.0)
            for ci in range(1, min(prefetch, nch)):
                staged[ci] = stage_chunk(ci)

            # target-logit work for n-tile i: one fused DVE mul+reduce
            def tgt_work(i):
                xa2 = sp.tile([128, d], bf16, tag="xa2", name="xa2")
                nc.sync.dma_start(xa2[:], x_nat[i * 128:(i + 1) * 128, :])
                wla = sp.tile([128, d], bf16, tag="wla", name="wla")
                nc.sync.dma_start(wla[:], wl[i * 128:(i + 1) * 128, :])
                pr = scp.tile([128, d], f32, tag="pr", name="pr")
                if use_ttr:
                    nc.vector.tensor_tensor_reduce(
                        out=pr[:], in0=xa2[:], in1=wla[:], scale=1.0,
                        scalar=0.0, op0=ALU.mult, op1=ALU.add,
                        accum_out=tgt[:, i:i + 1])
                else:
                    nc.vector.tensor_mul(pr[:], xa2[:], wla[:])
                    nc.vector.reduce_sum(tgt[:, i:i + 1], pr[:], axis=AX.X)

            # main loop: 49 chunks x 4 n-tiles. One 4-bank PSUM group per
            # (chunk, i); ScalarE consumes banks 0-2 (exact exp, in-place,
            # fused accum), VectorE consumes bank 3 (fast-exp + reduce).
            # two separate PSUM pools so the ACT (banks 0-2) and DVE
            # (bank 3) consumers are independent tiles -- a single 4-bank
            # tile made the scheduler serialize the DVE read behind the
            # ACT accumulator-read, stalling the PE ~1.1us every 2 groups
            tgt_done = set()
            with (
                tc.tile_pool(name="psumA", bufs=2, space="PSUM") as psa,
                tc.tile_pool(name="psumD", bufs=2, space="PSUM") as psd,
            ):
                for ci in range(nch):
                    wb = staged.pop(ci)
                    if ci + prefetch < nch:
                        staged[ci + prefetch] = stage_chunk(ci + prefetch)
                    cw = chunks[ci][2]
                    aw = min(act_w, cw)
                    njc = (cw + ct - 1) // ct
                    for i in range(ni):
                        ps = psa.tile([128, 3 * ct], f32, tag="ps",
                                      name="ps")
                        pd = psd.tile([128, ct], f32, tag="pd", name="pd")
                        for g in range(nk8):
                            lhs = xtb[g][:, :, i * 128:(i + 1) * 128]
                            for jc in range(njc):
                                w0, w1 = jc * ct, min((jc + 1) * ct, cw)
                                if rhs_sliced == 2:
                                    rhs = wb[(g, jc)][:, :, :w1 - w0]
                                elif rhs_sliced:
                                    rhs = wb[:, 2 * g:2 * g + 2, w0:w1]
                                else:
                                    rhs = wb[g][:, :, w0:w1]
                                dst = (ps[:, w0:w1] if jc < 3
                                       else pd[:, :w1 - w0])
                                nc.tensor.matmul(
                                    dst, lhs, rhs,
                                    start=(g == 0), stop=(g == nk8 - 1),
                                    perf_mode=(
                                        mybir.MatmulPerfMode.DoubleRow))
                        col = 2 * (i * nch + ci)
                        if inplace:
                            act_dst = ps[:, :aw]
                        else:
                            es = scp.tile([128, 3 * ct], bf16, tag="es",
                                          name="es")
                            act_dst = es[:, :aw]
                        nc.scalar.activation(
                            act_dst, ps[:, :aw], AF.Exp,
                            scale=a_all[:, i:i + 1],
                            accum_out=parts[:, col:col + 1])
                        dw = cw - aw
                        if dw > 0:
                            ti = scp.tile([128, ct], i32, tag="ti",
                                          name="ti")
                            nc.vector.tensor_scalar(
                                out=ti[:, :dw], in0=pd[:, :dw],
                                scalar1=a2_all[:, i:i + 1], scalar2=EXP_B,
                                op0=ALU.mult, op1=ALU.add)
                            nc.vector.reduce_sum(parts[:, col + 1:col + 2],
                                                 ti[:, :dw].bitcast(f32),
                                                 axis=AX.X)
                        else:
                            nc.vector.memset(parts[:, col + 1:col + 2], 0.0)
                    # spread the 4 tgt tiles across the loop interior
                    step = max(nch // (ni + 1), 1)
                    if ci % step == 0 and 1 <= ci // step <= ni \
                            and ci // step - 1 not in tgt_done:
                        tgt_work(ci // step - 1)
                        tgt_done.add(ci // step - 1)
                for i in range(ni):
                    if i not in tgt_done:
                        tgt_work(i)

            # epilogue: per-core partial = sum_i (log den_i - s*tgt_i)
            t1 = pp.tile([128, ni], f32, tag="t1", name="t1")
            e1 = pp.tile([128, ni], f32, tag="e1", name="e1")
            e2 = pp.tile([128, ni], f32, tag="e2", name="e2")
            loc = pp.tile([128, ni], f32, tag="loc", name="loc")
            den = pp.tile([128, ni], f32, tag="den", name="den")
            lg = pp.tile([128, ni], f32, tag="lg", name="lg")
            v = pp.tile([128, ni], f32, tag="v", name="v")
            rowv = pp.tile([128, 1], f32, tag="rowv", name="rowv")
            res = pp.tile([1, 1], f32, tag="res", name="res")

            for i in range(ni):
                nc.vector.reduce_sum(
                    loc[:, i:i + 1],
                    parts[:, 2 * i * nch:2 * (i + 1) * nch], axis=AX.X)
            nc.vector.tensor_mul(t1[:], a_all[:], tgt[:])   # s * tgt cosine
            nc.scalar.activation(e2[:], t1[:], AF.Exp)      # same table set
            nc.vector.tensor_scalar_mul(e1[:], e2[:], float(np.exp(-SM)))
            nc.vector.tensor_sub(e1[:], e1[:], e2[:])       # e^(t1-SM)-e^t1
            nc.vector.tensor_add(den[:], loc[:], e1[:])
            if use_fastlog:
                # fast-log: lg = (bits(den) - B) * ln2/2^23
                nc.vector.tensor_scalar(out=lg[:], in0=den[:].bitcast(i32),
                                        scalar1=LOG_K, scalar2=-LOG_B * LOG_K,
                                        op0=ALU.mult, op1=ALU.add)
            else:
                nc.scalar.activation(lg[:], den[:], AF.Ln)
            nc.vector.tensor_sub(v[:], lg[:], t1[:])
            nc.vector.reduce_sum(rowv[:], v[:], axis=AX.X)
            with tc.tile_pool(name="psum1", bufs=1, space="PSUM") as psp1:
                pss = psp1.tile([1, 1], f32, tag="pss", name="pss")
                nc.tensor.matmul(pss[:], rowv[:], ones[:], start=True,
                                 stop=True)
                nc.vector.tensor_scalar_mul(res[:], pss[:], 1.0)
            nc.sync.dma_start(out[:], res[:])

    nc.compile()
    return nc


def in_maps(x, W, labels, n_cores=N_CORES):
    ns = x.shape[0] // n_cores
    x = np.ascontiguousarray(np.asarray(x, dtype=np.float32))
    W = np.ascontiguousarray(np.asarray(W, dtype=np.float32))
    lab = np.asarray(labels).astype(np.int64)
    c, d = W.shape
    nch = (c + 2048 - 1) // 2048
    wtf = W.T.astype(np_fp8)                            # [D, C]
    wt = np.zeros((nch * d, 2048), np_fp8)              # chunk-major
    for ci in range(nch):
        cw = min(2048, c - ci * 2048)
        wt[ci * d:(ci + 1) * d, :cw] = wtf[:, ci * 2048:ci * 2048 + cw]
    wlg = np.ascontiguousarray(W[lab].astype(np_bf16))  # [N, D]
    maps = []
    for cid in range(n_cores):
        xs = x[cid * ns:(cid + 1) * ns]
        maps.append({
            "x_nat": np.ascontiguousarray(xs.astype(np_bf16)),
            "xtb": np.ascontiguousarray(xs.T.astype(np_fp8)),
            "wl": np.ascontiguousarray(wlg[cid * ns:(cid + 1) * ns]),
            "wt": wt,
        })
    return maps


def gather(results, n=N):
    """Host-side unshard: mean over the per-core partial sums + margin."""
    tot = sum(float(np.asarray(r["out"], dtype=np.float32).reshape(()))
              for r in results)
    return np.float32(tot / n + SM)


_CACHE = {}


def _get_nc():
    if "nc" not in _CACHE:
        _CACHE["nc"] = build(inplace=0, prefetch=4)
    return _CACHE["nc"]


def kernel(x, W, labels):
    nc = _get_nc()
    res = run_bass_kernel_spmd(nc, in_maps(x, W, labels),
                               core_ids=list(range(N_CORES)))
    return gather(res.results).reshape(())


# revision 39
# speedup vs baseline: 1.1077x; 1.1077x over previous
"""AngularPenaltySMLoss (CosFace) on 8 TRN2 NeuronCores.

Strategy: data-parallel over the batch N=4096. Each core owns 512 samples
and computes the FULL class dimension C=100000 for them, so no collective
is needed: each core emits its partial sum of (log den_i - s*tgt_i) and
the host sums the 8 partials (the mean + margin fold is host-side too).

Per core, per (n-tile i, chunk of 4 c-tiles):
  - logits [128 n x <=2048 c] = fp8 DoubleRow matmuls, xT stationary,
    W^T moving, K=512 contracted as 2 accumulating 256-row steps into a
    4-bank PSUM group.
  - consumer split so neither engine exceeds the ~2.07us of PE work per
    group: ScalarE takes banks 0-2 (exact Exp, per-partition scale
    a[n] = S/||x_n||, fused row-sum accumulator); VectorE takes bank 3
    via the Schraudolph fast-exp
    bit trick + row reduce. No zero padding: the last c-tile is 160 wide.
  - norms via DVE only (squares fused mul+reduce, rsqrt via quake bit
    trick + 2 Newton steps) so ScalarE never loads the Sqrt table set.
  - target logits from host-gathered W[labels] rows: one fused DVE
    mul+reduce per n-tile, interleaved mid-loop.
  - epilogue log via the inverse-Schraudolph bit trick on DVE (no Ln
    table load); the only ACT table set ever loaded is Exp's.

W^T is cast to fp8 once on the host and shared by all 8 cores (full C).
"""

import ml_dtypes
import numpy as np

from concourse import bacc, mybir, tile
from concourse.bass_utils import run_bass_kernel_spmd

N, D, C = 4096, 512, 100000
N_CORES = 8
NS = N // N_CORES               # 512 samples per core
S = 30.0
SM = 10.5                       # S * margin(0.35)
CT = 512                        # c-tile width (one PSUM bank of f32)
NCH = (C + 4 * CT - 1) // (4 * CT)   # 49 chunks of up to 4 banks

# Schraudolph fast-exp constants (DVE offload): exp(x) ~= bitcast_f32(
# int32(x * 2^23/ln2 + (127*2^23 - C))), C=486411 zeroes the mean error
EXP_A = float(2 ** 23 / np.log(2))
EXP_B = float(1065353216 - 486411)
# inverse (fast-log): ln(x) ~= (bitcast_i32(x) - B) * ln2/2^23,
# B = 2^23*(127 - 0.0430357) zeroes the mean error
LOG_K = float(np.log(2) / 2 ** 23)
LOG_B = 1065353216.0 - round(2 ** 23 * 0.0430357)
RSQ_MAGIC = 1597463007.0        # 0x5f3759df quake rsqrt seed

f32 = mybir.dt.float32
bf16 = mybir.dt.bfloat16
fp8 = mybir.dt.float8e4
i32 = mybir.dt.int32
np_bf16 = ml_dtypes.bfloat16
np_fp8 = mybir.dt.np(mybir.dt.float8e4)
AF = mybir.ActivationFunctionType
ALU = mybir.AluOpType
AX = mybir.AxisListType


def build(ns=NS, d=D, c=C, ct=CT, n_cores=N_CORES, act_w=1536, inplace=1,
          prefetch=3, use_ttr=0, use_quake=1, use_fastlog=1, rhs_sliced=1,
          split=1):
    # use_ttr=1 (InstTensorTensorReduce) crashes real HW (NRT INTERNAL)
    # even though CoreSim accepts it -- probed 2026-08-07; keep it off.
    ni = ns // 128                 # 4 n-tiles
    nk8 = d // 256                 # 2 DoubleRow K-steps
    nhb = (c + 4 * ct - 1) // (4 * ct)   # host 2048-wide row blocks
    # chunk descriptors (host_block, col_off, width), one per host block
    # (splitting the first block into single-bank chunks was tried and
    # regressed: tiny groups are consumer-limited, ~1.05us for 0.43us of
    # PE work)
    chunks = [(hb, 0, min(4 * ct, c - 4 * ct * hb)) for hb in range(nhb)]
    nch = len(chunks)

    nc = bacc.Bacc("TRN2", target_bir_lowering=False, debug=False,
                   num_devices=n_cores)
    x_nat = nc.dram_tensor("x_nat", [ns, d], bf16, kind="ExternalInput").ap()
    xtb_d = nc.dram_tensor("xtb", [d, ns], fp8, kind="ExternalInput").ap()
    wl = nc.dram_tensor("wl", [ns, d], bf16, kind="ExternalInput").ap()
    # W^T stored chunk-major ([nch*d, 4*ct], last chunk zero-padded) so
    # every DMA stride stays small (the flat [d, C] layout would need a
    # 100000-byte partition stride)
    wt = nc.dram_tensor("wt", [nhb * d, 4 * ct], fp8,
                        kind="ExternalInput").ap()
    out = nc.dram_tensor("out", [1, 1], f32, kind="ExternalOutput").ap()

    with tile.TileContext(nc) as tc:
        with (
            tc.tile_pool(name="persist", bufs=1) as pp,
            tc.tile_pool(name="stage", bufs=3) as sp,
            tc.tile_pool(name="wbuf",
                         bufs=(prefetch + 1) * (8 if rhs_sliced == 2 else 1)
                         ) as wbp,
            tc.tile_pool(name="scr", bufs=2) as scp,
        ):
            xtb = [pp.tile([128, 2, ns], fp8, tag=f"xtb{g}",
                           name=f"xtbs{g}") for g in range(nk8)]
            parts = pp.tile([128, ni * nch * 2], f32, tag="parts",
                            name="parts")
            ss = pp.tile([128, ni], f32, tag="ss", name="ss")
            tgt = pp.tile([128, ni], f32, tag="tgt", name="tgt")
            a_all = pp.tile([128, ni], f32, tag="a_all", name="a_all")
            a2_all = pp.tile([128, ni], f32, tag="a2_all", name="a2_all")
            ones = pp.tile([128, 1], f32, tag="ones", name="ones")

            # xT resident in SBUF -- gates the first matmuls
            for g in range(nk8):
                nc.sync.dma_start(
                    xtb[g][:],
                    xtb_d[g * 256:(g + 1) * 256, :].rearrange(
                        "(s p) n -> p s n", s=2))

            # W-chunk staging: one wide DMA per chunk, 4 DoubleRow k-pair
            # planes so rhs slices [:, 2g:2g+2, :] feed the matmuls
            def stage_chunk(ci, fine=False):
                hb, c0, cw = chunks[ci]
                rows = wt[hb * d:(hb + 1) * d, c0:c0 + cw]
                if rhs_sliced == 2:
                    # exact baseline staging: per-(g, jc) [128, 2, ct]
                    # tiles, rhs APs are whole tiles
                    wbt = {}
                    for jc in range((cw + ct - 1) // ct):
                        w0, w1 = jc * ct, min((jc + 1) * ct, cw)
                        for g in range(nk8):
                            wb = wbp.tile([128, 2, ct], fp8, tag="wbe",
                                          name="wbe")
                            nc.sync.dma_start(
                                wb[:, :, :w1 - w0],
                                rows[g * 256:(g + 1) * 256, w0:w1].rearrange(
                                    "(s p) c -> p s c", s=2))
                            wbt[(g, jc)] = wb
                    return wbt
                if not rhs_sliced:
                    # baseline-shaped staging: one [128, 2, cw] tile per
                    # DoubleRow k-group, rhs APs never slice the pair dim
                    wbg = []
                    for g in range(nk8):
                        wb = wbp.tile([128, 2, 4 * ct], fp8, tag=f"wbg{g}",
                                      name=f"wbg{g}")
                        nc.sync.dma_start(
                            wb[:, :, :cw],
                            rows[g * 256:(g + 1) * 256, :cw].rearrange(
                                "(s p) c -> p s c", s=2))
                        wbg.append(wb)
                    return wbg
                wb = wbp.tile([128, 4, 4 * ct], fp8, tag="wb", name="wb")
                if fine:   # per-bank DMAs so the first matmul starts ASAP
                    for jc in range((cw + ct - 1) // ct):
                        w0, w1 = jc * ct, min((jc + 1) * ct, cw)
                        nc.sync.dma_start(
                            wb[:, :, w0:w1],
                            rows[:, w0:w1].rearrange("(s p) c -> p s c", s=4))
                else:
                    nc.sync.dma_start(
                        wb[:, :, :cw],
                        rows[:, :cw].rearrange("(s p) c -> p s c", s=4))
                return wb

            # chunk0 first (gates the first matmuls), then the norm x
            # tiles (gate the first ACT at ~t+4us), then the deeper W
            # prefetch -- all squeezed into the same ~300GB/s DMA stream
            staged = {0: stage_chunk(0, fine=True)}

            # norms, DVE only: ss = sum(x^2)/S^2, a = rsqrt(ss), then the
            # Schraudolph pre-scale a2 = a * 2^23/ln2
            for i in range(ni):
                xa = sp.tile([128, d], bf16, tag="xa", name="xa")
                nc.sync.dma_start(xa[:], x_nat[i * 128:(i + 1) * 128, :])
                sq = scp.tile([128, d], f32, tag="sq", name="sq")
                if use_ttr:
                    nc.vector.tensor_tensor_reduce(
                        out=sq[:], in0=xa[:], in1=xa[:], scale=1.0 / (S * S),
                        scalar=0.0, op0=ALU.mult, op1=ALU.add,
                        accum_out=ss[:, i:i + 1])
                else:
                    nc.vector.tensor_mul(sq[:], xa[:], xa[:])
                    nc.vector.reduce_sum(ss[:, i:i + 1], sq[:], axis=AX.X)
            if not use_ttr:
                # fold the S factor: a = rsqrt(ss / S^2) = S / ||x||
                nc.vector.tensor_scalar_mul(ss[:], ss[:], 1.0 / (S * S))
            if use_quake:
                y0i = pp.tile([128, ni], i32, tag="y0i", name="y0i")
                yt = pp.tile([128, ni], f32, tag="yt", name="yt")
                rt = pp.tile([128, ni], f32, tag="rt", name="rt")
                # seed: bits(y0) = MAGIC - bits(ss)/2 (int arithmetic done
                # in f32; the low bits it rounds away are noise the Newton
                # steps absorb)
                nc.vector.tensor_scalar(out=y0i[:], in0=ss[:].bitcast(i32),
                                        scalar1=-0.5, scalar2=RSQ_MAGIC,
                                        op0=ALU.mult, op1=ALU.add)
                ycur = y0i[:].bitcast(f32)
                for it in range(2):
                    dst_y = a_all if it == 1 else yt
                    nc.vector.tensor_mul(rt[:], ycur, ycur)
                    nc.vector.tensor_mul(rt[:], rt[:], ss[:])
                    nc.vector.tensor_scalar(out=rt[:], in0=rt[:],
                                            scalar1=-0.5, scalar2=1.5,
                                            op0=ALU.mult, op1=ALU.add)
                    nc.vector.tensor_mul(dst_y[:], ycur, rt[:])
                    ycur = dst_y[:]
            else:
                ut = pp.tile([128, ni], f32, tag="ut", name="ut")
                nc.scalar.activation(ut[:], ss[:], AF.Sqrt)
                nc.vector.reciprocal(a_all[:], ut[:])
            nc.vector.tensor_scalar_mul(a2_all[:], a_all[:], EXP_A)
            nc.vector.memset(ones[:], 1.0)
            for ci in range(1, min(prefetch, nch)):
                staged[ci] = stage_chunk(ci)

            # target-logit work for n-tile i: one fused DVE mul+reduce
            def tgt_work(i):
                xa2 = sp.tile([128, d], bf16, tag="xa2", name="xa2")
                nc.sync.dma_start(xa2[:], x_nat[i * 128:(i + 1) * 128, :])
                wla = sp.tile([128, d], bf16, tag="wla", name="wla")
                nc.sync.dma_start(wla[:], wl[i * 128:(i + 1) * 128, :])
                pr = scp.tile([128, d], f32, tag="pr", name="pr")
                if use_ttr:
                    nc.vector.tensor_tensor_reduce(
                        out=pr[:], in0=xa2[:], in1=wla[:], scale=1.0,
                        scalar=0.0, op0=ALU.mult, op1=ALU.add,
                        accum_out=tgt[:, i:i + 1])
                else:
                    nc.vector.tensor_mul(pr[:], xa2[:], wla[:])
                    nc.vector.reduce_sum(tgt[:, i:i + 1], pr[:], axis=AX.X)

            # main loop: 49 chunks x 4 n-tiles. One 4-bank PSUM group per
            # (chunk, i); ScalarE consumes banks 0-2 (exact exp, in-place,
            # fused accum), VectorE consumes bank 3 (fast-exp + reduce).
            # two separate PSUM pools so the ACT (banks 0-2) and DVE
            # (bank 3) consumers are independent tiles -- a single 4-bank
            # tile made the scheduler serialize the DVE read behind the
            # ACT accumulator-read, stalling the PE ~1.1us every 2 groups
            tgt_done = set()
            with (
                tc.tile_pool(name="psumA", bufs=2, space="PSUM") as psa,
                tc.tile_pool(name="psumD", bufs=2, space="PSUM") as psd,
            ):
                for ci in range(nch):
                    wb = staged.pop(ci)
                    if ci + prefetch < nch:
                        staged[ci + prefetch] = stage_chunk(ci + prefetch)
                    cw = chunks[ci][2]
                    aw = min(act_w, cw)
                    njc = (cw + ct - 1) // ct
                    for i in range(ni):
                        ps = psa.tile([128, 3 * ct], f32, tag="ps",
                                      name="ps")
                        pd = psd.tile([128, ct], f32, tag="pd", name="pd")
                        for g in range(nk8):
                            lhs = xtb[g][:, :, i * 128:(i + 1) * 128]
                            for jc in range(njc):
                                w0, w1 = jc * ct, min((jc + 1) * ct, cw)
                                if rhs_sliced == 2:
                                    rhs = wb[(g, jc)][:, :, :w1 - w0]
                                elif rhs_sliced:
                                    rhs = wb[:, 2 * g:2 * g + 2, w0:w1]
                                else:
                                    rhs = wb[g][:, :, w0:w1]
                                dst = (ps[:, w0:w1] if jc < 3
                                       else pd[:, :w1 - w0])
                                nc.tensor.matmul(
                                    dst, lhs, rhs,
                                    start=(g == 0), stop=(g == nk8 - 1),
                                    perf_mode=(
                                        mybir.MatmulPerfMode.DoubleRow))
                        col = 2 * (i * nch + ci)
                        if inplace:
                            act_dst = ps[:, :aw]
                        else:
                            es = scp.tile([128, 3 * ct], bf16, tag="es",
                                          name="es")
                            act_dst = es[:, :aw]
                        nc.scalar.activation(
                            act_dst, ps[:, :aw], AF.Exp,
                            scale=a_all[:, i:i + 1],
                            accum_out=parts[:, col:col + 1])
                        dw = cw - aw
                        if dw > 0:
                            ti = scp.tile([128, ct], i32, tag="ti",
                                          name="ti")
                            nc.vector.tensor_scalar(
                                out=ti[:, :dw], in0=pd[:, :dw],
                                scalar1=a2_all[:, i:i + 1], scalar2=EXP_B,
                                op0=ALU.mult, op1=ALU.add)
                            nc.vector.reduce_sum(parts[:, col + 1:col + 2],
                                                 ti[:, :dw].bitcast(f32),
                                                 axis=AX.X)
                        else:
                            nc.vector.memset(parts[:, col + 1:col + 2], 0.0)
                    # spread the 4 tgt tiles across the loop interior
                    step = max(nch // (ni + 1), 1)
                    if ci % step == 0 and 1 <= ci // step <= ni \
                            and ci // step - 1 not in tgt_done:
                        tgt_work(ci // step - 1)
                        tgt_done.add(ci // step - 1)
                for i in range(ni):
                    if i not in tgt_done:
                        tgt_work(i)

            # epilogue: per-core partial = sum_i (log den_i - s*tgt_i)
            t1 = pp.tile([128, ni], f32, tag="t1", name="t1")
            e1 = pp.tile([128, ni], f32, tag="e1", name="e1")
            e2 = pp.tile([128, ni], f32, tag="e2", name="e2")
            loc = pp.tile([128, ni], f32, tag="loc", name="loc")
            den = pp.tile([128, ni], f32, tag="den", name="den")
            lg = pp.tile([128, ni], f32, tag="lg", name="lg")
            v = pp.tile([128, ni], f32, tag="v", name="v")
            rowv = pp.tile([128, 1], f32, tag="rowv", name="rowv")
            res = pp.tile([1, 1], f32, tag="res", name="res")

            for i in range(ni):
                nc.vector.reduce_sum(
                    loc[:, i:i + 1],
                    parts[:, 2 * i * nch:2 * (i + 1) * nch], axis=AX.X)
            nc.vector.tensor_mul(t1[:], a_all[:], tgt[:])   # s * tgt cosine
            nc.scalar.activation(e2[:], t1[:], AF.Exp)      # same table set
            nc.vector.tensor_scalar_mul(e1[:], e2[:], float(np.exp(-SM)))
            nc.vector.tensor_sub(e1[:], e1[:], e2[:])       # e^(t1-SM)-e^t1
            nc.vector.tensor_add(den[:], loc[:], e1[:])
            if use_fastlog:
                # fast-log: lg = (bits(den) - B) * ln2/2^23
                nc.vector.tensor_scalar(out=lg[:], in0=den[:].bitcast(i32),
                                        scalar1=LOG_K, scalar2=-LOG_B * LOG_K,
                                        op0=ALU.mult, op1=ALU.add)
            else:
                nc.scalar.activation(lg[:], den[:], AF.Ln)
            nc.vector.tensor_sub(v[:], lg[:], t1[:])
            nc.vector.reduce_sum(rowv[:], v[:], axis=AX.X)
            with tc.tile_pool(name="psum1", bufs=1, space="PSUM") as psp1:
                pss = psp1.tile([1, 1], f32, tag="pss", name="pss")
                nc.tensor.matmul(pss[:], rowv[:], ones[:], start=True,
                                 stop=True)
                nc.vector.tensor_scalar_mul(res[:], pss[:], 1.0)
            nc.sync.dma_start(out[:], res[:])

    nc.compile()
    return nc


def in_maps(x, W, labels, n_cores=N_CORES):
    ns = x.shape[0] // n_cores
    x = np.ascontiguousarray(np.asarray(x, dtype=np.float32))
    W = np.ascontiguousarray(np.asarray(W, dtype=np.float32))
    lab = np.asarray(labels).astype(np.int64)
    c, d = W.shape
    nch = (c + 2048 - 1) // 2048
    wtf = W.T.astype(np_fp8)                            # [D, C]
    wt = np.zeros((nch * d, 2048), np_fp8)              # chunk-major
    for ci in range(nch):
        cw = min(2048, c - ci * 2048)
        wt[ci * d:(ci + 1) * d, :cw] = wtf[:, ci * 2048:ci * 2048 + cw]
    wlg = np.ascontiguousarray(W[lab].astype(np_bf16))  # [N, D]
    maps = []
    for cid in range(n_cores):
        xs = x[cid * ns:(cid + 1) * ns]
        maps.append({
            "x_nat": np.ascontiguousarray(xs.astype(np_bf16)),
            "xtb": np.ascontiguousarray(xs.T.astype(np_fp8)),
            "wl": np.ascontiguousarray(wlg[cid * ns:(cid + 1) * ns]),
            "wt": wt,
        })
    return maps


def gather(results, n=N):
    """Host-side unshard: mean over the per-core partial sums + margin."""
    tot = sum(float(np.asarray(r["out"], dtype=np.float32).reshape(()))
              for r in results)
    return np.float32(tot / n + SM)


_CACHE = {}


def _get_nc():
    if "nc" not in _CACHE:
        _CACHE["nc"] = build(inplace=0, prefetch=7)
    return _CACHE["nc"]


def kernel(x, W, labels):
    nc = _get_nc()
    res = run_bass_kernel_spmd(nc, in_maps(x, W, labels),
                               core_ids=list(range(N_CORES)))
    return gather(res.results).reshape(())


# revision 40
# speedup vs baseline: 1.1286x; 1.0189x over previous
"""AngularPenaltySMLoss (CosFace) on 8 TRN2 NeuronCores.

Strategy: data-parallel over the batch N=4096. Each core owns 512 samples
and computes the FULL class dimension C=100000 for them, so no collective
is needed: each core emits its partial sum of (log den_i - s*tgt_i) and
the host sums the 8 partials (the mean + margin fold is host-side too).

Per core, per (n-tile i, chunk of 4 c-tiles):
  - logits [128 n x <=2048 c] = fp8 DoubleRow matmuls, xT stationary,
    W^T moving, K=512 contracted as 2 accumulating 256-row steps into a
    4-bank PSUM group.
  - consumer split so neither engine exceeds the ~2.07us of PE work per
    group: ScalarE takes banks 0-2 (exact Exp, per-partition scale
    a[n] = S/||x_n||, fused row-sum accumulator); VectorE takes bank 3
    via the Schraudolph fast-exp
    bit trick + row reduce. No zero padding: the last c-tile is 160 wide.
  - norms via DVE only (squares fused mul+reduce, rsqrt via quake bit
    trick + 2 Newton steps) so ScalarE never loads the Sqrt table set.
  - target logits from host-gathered W[labels] rows: one fused DVE
    mul+reduce per n-tile, interleaved mid-loop.
  - epilogue log via the inverse-Schraudolph bit trick on DVE (no Ln
    table load); the only ACT table set ever loaded is Exp's.

W^T is cast to fp8 once on the host and shared by all 8 cores (full C).
"""

import ml_dtypes
import numpy as np

from concourse import bacc, mybir, tile
from concourse.bass_utils import run_bass_kernel_spmd

N, D, C = 4096, 512, 100000
N_CORES = 8
NS = N // N_CORES               # 512 samples per core
S = 30.0
SM = 10.5                       # S * margin(0.35)
CT = 512                        # c-tile width (one PSUM bank of f32)
NCH = (C + 4 * CT - 1) // (4 * CT)   # 49 chunks of up to 4 banks

# Schraudolph fast-exp constants (DVE offload): exp(x) ~= bitcast_f32(
# int32(x * 2^23/ln2 + (127*2^23 - C))), C=486411 zeroes the mean error
EXP_A = float(2 ** 23 / np.log(2))
EXP_B = float(1065353216 - 486411)
# inverse (fast-log): ln(x) ~= (bitcast_i32(x) - B) * ln2/2^23,
# B = 2^23*(127 - 0.0430357) zeroes the mean error
LOG_K = float(np.log(2) / 2 ** 23)
LOG_B = 1065353216.0 - round(2 ** 23 * 0.0430357)
RSQ_MAGIC = 1597463007.0        # 0x5f3759df quake rsqrt seed

f32 = mybir.dt.float32
bf16 = mybir.dt.bfloat16
fp8 = mybir.dt.float8e4
i32 = mybir.dt.int32
np_bf16 = ml_dtypes.bfloat16
np_fp8 = mybir.dt.np(mybir.dt.float8e4)
AF = mybir.ActivationFunctionType
ALU = mybir.AluOpType
AX = mybir.AxisListType


def build(ns=NS, d=D, c=C, ct=CT, n_cores=N_CORES, act_w=1536, inplace=1,
          prefetch=3, use_ttr=0, use_quake=1, use_fastlog=1, rhs_sliced=1,
          split=1):
    # use_ttr=1 (InstTensorTensorReduce) crashes real HW (NRT INTERNAL)
    # even though CoreSim accepts it -- probed 2026-08-07; keep it off.
    ni = ns // 128                 # 4 n-tiles
    nk8 = d // 256                 # 2 DoubleRow K-steps
    nhb = (c + 4 * ct - 1) // (4 * ct)   # host 2048-wide row blocks
    # chunk descriptors (host_block, col_off, width), one per host block
    # (splitting the first block into single-bank chunks was tried and
    # regressed: tiny groups are consumer-limited, ~1.05us for 0.43us of
    # PE work)
    chunks = [(hb, 0, min(4 * ct, c - 4 * ct * hb)) for hb in range(nhb)]
    nch = len(chunks)

    nc = bacc.Bacc("TRN2", target_bir_lowering=False, debug=False,
                   num_devices=n_cores)
    x_nat = nc.dram_tensor("x_nat", [ns, d], bf16, kind="ExternalInput").ap()
    xtb_d = nc.dram_tensor("xtb", [d, ns], fp8, kind="ExternalInput").ap()
    wl = nc.dram_tensor("wl", [ns, d], bf16, kind="ExternalInput").ap()
    # W^T stored chunk-major ([nch*d, 4*ct], last chunk zero-padded) so
    # every DMA stride stays small (the flat [d, C] layout would need a
    # 100000-byte partition stride)
    wt = nc.dram_tensor("wt", [nhb * d, 4 * ct], fp8,
                        kind="ExternalInput").ap()
    out = nc.dram_tensor("out", [1, 1], f32, kind="ExternalOutput").ap()

    with tile.TileContext(nc) as tc:
        with (
            tc.tile_pool(name="persist", bufs=1) as pp,
            tc.tile_pool(name="stage", bufs=3) as sp,
            tc.tile_pool(name="wbuf",
                         bufs=(prefetch + 1) * (8 if rhs_sliced == 2 else 1)
                         ) as wbp,
            tc.tile_pool(name="scr", bufs=2) as scp,
        ):
            xtb = [pp.tile([128, 2, ns], fp8, tag=f"xtb{g}",
                           name=f"xtbs{g}") for g in range(nk8)]
            parts = pp.tile([128, ni * nch * 2], f32, tag="parts",
                            name="parts")
            ss = pp.tile([128, ni], f32, tag="ss", name="ss")
            tgt = pp.tile([128, ni], f32, tag="tgt", name="tgt")
            a_all = pp.tile([128, ni], f32, tag="a_all", name="a_all")
            a2_all = pp.tile([128, ni], f32, tag="a2_all", name="a2_all")
            ones = pp.tile([128, 1], f32, tag="ones", name="ones")

            # xT resident in SBUF -- gates the first matmuls
            for g in range(nk8):
                nc.sync.dma_start(
                    xtb[g][:],
                    xtb_d[g * 256:(g + 1) * 256, :].rearrange(
                        "(s p) n -> p s n", s=2))

            # W-chunk staging: one wide DMA per chunk, 4 DoubleRow k-pair
            # planes so rhs slices [:, 2g:2g+2, :] feed the matmuls
            def stage_chunk(ci, fine=False):
                hb, c0, cw = chunks[ci]
                rows = wt[hb * d:(hb + 1) * d, c0:c0 + cw]
                if rhs_sliced == 2:
                    # exact baseline staging: per-(g, jc) [128, 2, ct]
                    # tiles, rhs APs are whole tiles
                    wbt = {}
                    for jc in range((cw + ct - 1) // ct):
                        w0, w1 = jc * ct, min((jc + 1) * ct, cw)
                        for g in range(nk8):
                            wb = wbp.tile([128, 2, ct], fp8, tag="wbe",
                                          name="wbe")
                            nc.sync.dma_start(
                                wb[:, :, :w1 - w0],
                                rows[g * 256:(g + 1) * 256, w0:w1].rearrange(
                                    "(s p) c -> p s c", s=2))
                            wbt[(g, jc)] = wb
                    return wbt
                if not rhs_sliced:
                    # baseline-shaped staging: one [128, 2, cw] tile per
                    # DoubleRow k-group, rhs APs never slice the pair dim
                    wbg = []
                    for g in range(nk8):
                        wb = wbp.tile([128, 2, 4 * ct], fp8, tag=f"wbg{g}",
                                      name=f"wbg{g}")
                        nc.sync.dma_start(
                            wb[:, :, :cw],
                            rows[g * 256:(g + 1) * 256, :cw].rearrange(
                                "(s p) c -> p s c", s=2))
                        wbg.append(wb)
                    return wbg
                wb = wbp.tile([128, 4, 4 * ct], fp8, tag="wb", name="wb")
                if fine:   # per-bank DMAs so the first matmul starts ASAP
                    for jc in range((cw + ct - 1) // ct):
                        w0, w1 = jc * ct, min((jc + 1) * ct, cw)
                        nc.sync.dma_start(
                            wb[:, :, w0:w1],
                            rows[:, w0:w1].rearrange("(s p) c -> p s c", s=4))
                else:
                    nc.sync.dma_start(
                        wb[:, :, :cw],
                        rows[:, :cw].rearrange("(s p) c -> p s c", s=4))
                return wb

            # chunk0 first (gates the first matmuls), then the norm x
            # tiles (gate the first ACT at ~t+4us), then the deeper W
            # prefetch -- all squeezed into the same ~300GB/s DMA stream
            staged = {0: stage_chunk(0, fine=True)}

            # norms, DVE only: ss = sum(x^2)/S^2, a = rsqrt(ss), then the
            # Schraudolph pre-scale a2 = a * 2^23/ln2
            for i in range(ni):
                xa = sp.tile([128, d], bf16, tag="xa", name="xa")
                nc.sync.dma_start(xa[:], x_nat[i * 128:(i + 1) * 128, :])
                sq = scp.tile([128, d], f32, tag="sq", name="sq")
                if use_ttr:
                    nc.vector.tensor_tensor_reduce(
                        out=sq[:], in0=xa[:], in1=xa[:], scale=1.0 / (S * S),
                        scalar=0.0, op0=ALU.mult, op1=ALU.add,
                        accum_out=ss[:, i:i + 1])
                else:
                    nc.vector.tensor_mul(sq[:], xa[:], xa[:])
                    nc.vector.reduce_sum(ss[:, i:i + 1], sq[:], axis=AX.X)
            if not use_ttr:
                # fold the S factor: a = rsqrt(ss / S^2) = S / ||x||
                nc.vector.tensor_scalar_mul(ss[:], ss[:], 1.0 / (S * S))
            if use_quake:
                y0i = pp.tile([128, ni], i32, tag="y0i", name="y0i")
                yt = pp.tile([128, ni], f32, tag="yt", name="yt")
                rt = pp.tile([128, ni], f32, tag="rt", name="rt")
                # seed: bits(y0) = MAGIC - bits(ss)/2 (int arithmetic done
                # in f32; the low bits it rounds away are noise the Newton
                # steps absorb)
                nc.vector.tensor_scalar(out=y0i[:], in0=ss[:].bitcast(i32),
                                        scalar1=-0.5, scalar2=RSQ_MAGIC,
                                        op0=ALU.mult, op1=ALU.add)
                ycur = y0i[:].bitcast(f32)
                for it in range(2):
                    dst_y = a_all if it == 1 else yt
                    nc.vector.tensor_mul(rt[:], ycur, ycur)
                    nc.vector.tensor_mul(rt[:], rt[:], ss[:])
                    nc.vector.tensor_scalar(out=rt[:], in0=rt[:],
                                            scalar1=-0.5, scalar2=1.5,
                                            op0=ALU.mult, op1=ALU.add)
                    nc.vector.tensor_mul(dst_y[:], ycur, rt[:])
                    ycur = dst_y[:]
            else:
                ut = pp.tile([128, ni], f32, tag="ut", name="ut")
                nc.scalar.activation(ut[:], ss[:], AF.Sqrt)
                nc.vector.reciprocal(a_all[:], ut[:])
            nc.vector.tensor_scalar_mul(a2_all[:], a_all[:], EXP_A)
            nc.vector.memset(ones[:], 1.0)
            for ci in range(1, min(prefetch, nch)):
                staged[ci] = stage_chunk(ci)

            # target-logit work for n-tile i: one fused DVE mul+reduce
            def tgt_work(i):
                xa2 = sp.tile([128, d], bf16, tag="xa2", name="xa2")
                nc.sync.dma_start(xa2[:], x_nat[i * 128:(i + 1) * 128, :])
                wla = sp.tile([128, d], bf16, tag="wla", name="wla")
                nc.sync.dma_start(wla[:], wl[i * 128:(i + 1) * 128, :])
                pr = scp.tile([128, d], f32, tag="pr", name="pr")
                if use_ttr:
                    nc.vector.tensor_tensor_reduce(
                        out=pr[:], in0=xa2[:], in1=wla[:], scale=1.0,
                        scalar=0.0, op0=ALU.mult, op1=ALU.add,
                        accum_out=tgt[:, i:i + 1])
                else:
                    nc.vector.tensor_mul(pr[:], xa2[:], wla[:])
                    nc.vector.reduce_sum(tgt[:, i:i + 1], pr[:], axis=AX.X)

            # main loop: 49 chunks x 4 n-tiles. One 4-bank PSUM group per
            # (chunk, i); ScalarE consumes banks 0-2 (exact exp, in-place,
            # fused accum), VectorE consumes bank 3 (fast-exp + reduce).
            # two separate PSUM pools so the ACT (banks 0-2) and DVE
            # (bank 3) consumers are independent tiles -- a single 4-bank
            # tile made the scheduler serialize the DVE read behind the
            # ACT accumulator-read, stalling the PE ~1.1us every 2 groups
            tgt_done = set()
            with (
                tc.tile_pool(name="psumA", bufs=2, space="PSUM") as psa,
                tc.tile_pool(name="psumD", bufs=2, space="PSUM") as psd,
            ):
                for ci in range(nch):
                    wb = staged.pop(ci)
                    if ci + prefetch < nch:
                        staged[ci + prefetch] = stage_chunk(ci + prefetch)
                    cw = chunks[ci][2]
                    aw = min(act_w, cw)
                    njc = (cw + ct - 1) // ct
                    for i in range(ni):
                        ps = psa.tile([128, 3 * ct], f32, tag="ps",
                                      name="ps")
                        pd = psd.tile([128, ct], f32, tag="pd", name="pd")
                        for g in range(nk8):
                            lhs = xtb[g][:, :, i * 128:(i + 1) * 128]
                            for jc in range(njc):
                                w0, w1 = jc * ct, min((jc + 1) * ct, cw)
                                if rhs_sliced == 2:
                                    rhs = wb[(g, jc)][:, :, :w1 - w0]
                                elif rhs_sliced:
                                    rhs = wb[:, 2 * g:2 * g + 2, w0:w1]
                                else:
                                    rhs = wb[g][:, :, w0:w1]
                                dst = (ps[:, w0:w1] if jc < 3
                                       else pd[:, :w1 - w0])
                                nc.tensor.matmul(
                                    dst, lhs, rhs,
                                    start=(g == 0), stop=(g == nk8 - 1),
                                    perf_mode=(
                                        mybir.MatmulPerfMode.DoubleRow))
                        col = 2 * (i * nch + ci)
                        if inplace:
                            act_dst = ps[:, :aw]
                        else:
                            es = scp.tile([128, 3 * ct], bf16, tag="es",
                                          name="es")
                            act_dst = es[:, :aw]
                        nc.scalar.activation(
                            act_dst, ps[:, :aw], AF.Exp,
                            scale=a_all[:, i:i + 1],
                            accum_out=parts[:, col:col + 1])
                        dw = cw - aw
                        if dw > 0:
                            ti = scp.tile([128, ct], i32, tag="ti",
                                          name="ti")
                            nc.vector.tensor_scalar(
                                out=ti[:, :dw], in0=pd[:, :dw],
                                scalar1=a2_all[:, i:i + 1], scalar2=EXP_B,
                                op0=ALU.mult, op1=ALU.add)
                            nc.vector.reduce_sum(parts[:, col + 1:col + 2],
                                                 ti[:, :dw].bitcast(f32),
                                                 axis=AX.X)
                        else:
                            nc.vector.memset(parts[:, col + 1:col + 2], 0.0)
                    # spread the 4 tgt tiles across the loop interior
                    step = max(nch // (ni + 1), 1)
                    if ci % step == 0 and 1 <= ci // step <= ni \
                            and ci // step - 1 not in tgt_done:
                        tgt_work(ci // step - 1)
                        tgt_done.add(ci // step - 1)
                for i in range(ni):
                    if i not in tgt_done:
                        tgt_work(i)

            # epilogue: per-core partial = sum_i (log den_i - s*tgt_i)
            t1 = pp.tile([128, ni], f32, tag="t1", name="t1")
            e1 = pp.tile([128, ni], f32, tag="e1", name="e1")
            e2 = pp.tile([128, ni], f32, tag="e2", name="e2")
            loc = pp.tile([128, ni], f32, tag="loc", name="loc")
            den = pp.tile([128, ni], f32, tag="den", name="den")
            lg = pp.tile([128, ni], f32, tag="lg", name="lg")
            v = pp.tile([128, ni], f32, tag="v", name="v")
            rowv = pp.tile([128, 1], f32, tag="rowv", name="rowv")
            res = pp.tile([1, 1], f32, tag="res", name="res")

            for i in range(ni):
                nc.vector.reduce_sum(
                    loc[:, i:i + 1],
                    parts[:, 2 * i * nch:2 * (i + 1) * nch], axis=AX.X)
            nc.vector.tensor_mul(t1[:], a_all[:], tgt[:])   # s * tgt cosine
            nc.scalar.activation(e2[:], t1[:], AF.Exp)      # same table set
            nc.vector.tensor_scalar_mul(e1[:], e2[:], float(np.exp(-SM)))
            nc.vector.tensor_sub(e1[:], e1[:], e2[:])       # e^(t1-SM)-e^t1
            nc.vector.tensor_add(den[:], loc[:], e1[:])
            if use_fastlog:
                # fast-log: lg = (bits(den) - B) * ln2/2^23
                nc.vector.tensor_scalar(out=lg[:], in0=den[:].bitcast(i32),
                                        scalar1=LOG_K, scalar2=-LOG_B * LOG_K,
                                        op0=ALU.mult, op1=ALU.add)
            else:
                nc.scalar.activation(lg[:], den[:], AF.Ln)
            nc.vector.tensor_sub(v[:], lg[:], t1[:])
            nc.vector.reduce_sum(rowv[:], v[:], axis=AX.X)
            with tc.tile_pool(name="psum1", bufs=1, space="PSUM") as psp1:
                pss = psp1.tile([1, 1], f32, tag="pss", name="pss")
                nc.tensor.matmul(pss[:], rowv[:], ones[:], start=True,
                                 stop=True)
                nc.vector.tensor_scalar_mul(res[:], pss[:], 1.0)
            nc.sync.dma_start(out[:], res[:])

    nc.compile()
    return nc


def in_maps(x, W, labels, n_cores=N_CORES):
    ns = x.shape[0] // n_cores
    x = np.ascontiguousarray(np.asarray(x, dtype=np.float32))
    W = np.ascontiguousarray(np.asarray(W, dtype=np.float32))
    lab = np.asarray(labels).astype(np.int64)
    c, d = W.shape
    nch = (c + 2048 - 1) // 2048
    wtf = W.T.astype(np_fp8)                            # [D, C]
    wt = np.zeros((nch * d, 2048), np_fp8)              # chunk-major
    for ci in range(nch):
        cw = min(2048, c - ci * 2048)
        wt[ci * d:(ci + 1) * d, :cw] = wtf[:, ci * 2048:ci * 2048 + cw]
    wlg = np.ascontiguousarray(W[lab].astype(np_bf16))  # [N, D]
    maps = []
    for cid in range(n_cores):
        xs = x[cid * ns:(cid + 1) * ns]
        maps.append({
            "x_nat": np.ascontiguousarray(xs.astype(np_bf16)),
            "xtb": np.ascontiguousarray(xs.T.astype(np_fp8)),
            "wl": np.ascontiguousarray(wlg[cid * ns:(cid + 1) * ns]),
            "wt": wt,
        })
    return maps


def gather(results, n=N):
    """Host-side unshard: mean over the per-core partial sums + margin."""
    tot = sum(float(np.asarray(r["out"], dtype=np.float32).reshape(()))
              for r in results)
    return np.float32(tot / n + SM)


_CACHE = {}


def _get_nc():
    if "nc" not in _CACHE:
        _CACHE["nc"] = build(inplace=0, prefetch=10)
    return _CACHE["nc"]


def kernel(x, W, labels):
    nc = _get_nc()
    res = run_bass_kernel_spmd(nc, in_maps(x, W, labels),
                               core_ids=list(range(N_CORES)))
    return gather(res.results).reshape(())


# revision 41
# speedup vs baseline: 1.1632x; 1.0306x over previous
"""AngularPenaltySMLoss (CosFace) on 8 TRN2 NeuronCores.

Strategy: data-parallel over the batch N=4096. Each core owns 512 samples
and computes the FULL class dimension C=100000 for them, so no collective
is needed: each core emits its partial sum of (log den_i - s*tgt_i) and
the host sums the 8 partials (the mean + margin fold is host-side too).

Per core, per (n-tile i, chunk of 4 c-tiles):
  - logits [128 n x <=2048 c] = fp8 DoubleRow matmuls, xT stationary,
    W^T moving, K=512 contracted as 2 accumulating 256-row steps into a
    4-bank PSUM group.
  - consumer split so neither engine exceeds the ~2.07us of PE work per
    group: ScalarE takes banks 0-2 (exact Exp, per-partition scale
    a[n] = S/||x_n||, fused row-sum accumulator); VectorE takes bank 3
    via the Schraudolph fast-exp
    bit trick + row reduce. No zero padding: the last c-tile is 160 wide.
  - norms via DVE only (squares fused mul+reduce, rsqrt via quake bit
    trick + 2 Newton steps) so ScalarE never loads the Sqrt table set.
  - target logits from host-gathered W[labels] rows: one fused DVE
    mul+reduce per n-tile, interleaved mid-loop.
  - epilogue log via the inverse-Schraudolph bit trick on DVE (no Ln
    table load); the only ACT table set ever loaded is Exp's.

W^T is cast to fp8 once on the host and shared by all 8 cores (full C).
"""

import ml_dtypes
import numpy as np

from concourse import bacc, mybir, tile
from concourse.bass_utils import run_bass_kernel_spmd

N, D, C = 4096, 512, 100000
N_CORES = 8
NS = N // N_CORES               # 512 samples per core
S = 30.0
SM = 10.5                       # S * margin(0.35)
CT = 512                        # c-tile width (one PSUM bank of f32)
NCH = (C + 4 * CT - 1) // (4 * CT)   # 49 chunks of up to 4 banks

# Schraudolph fast-exp constants (DVE offload): exp(x) ~= bitcast_f32(
# int32(x * 2^23/ln2 + (127*2^23 - C))), C=486411 zeroes the mean error
EXP_A = float(2 ** 23 / np.log(2))
EXP_B = float(1065353216 - 486411)
# inverse (fast-log): ln(x) ~= (bitcast_i32(x) - B) * ln2/2^23,
# B = 2^23*(127 - 0.0430357) zeroes the mean error
LOG_K = float(np.log(2) / 2 ** 23)
LOG_B = 1065353216.0 - round(2 ** 23 * 0.0430357)
RSQ_MAGIC = 1597463007.0        # 0x5f3759df quake rsqrt seed

f32 = mybir.dt.float32
bf16 = mybir.dt.bfloat16
fp8 = mybir.dt.float8e4
i32 = mybir.dt.int32
np_bf16 = ml_dtypes.bfloat16
np_fp8 = mybir.dt.np(mybir.dt.float8e4)
AF = mybir.ActivationFunctionType
ALU = mybir.AluOpType
AX = mybir.AxisListType


def build(ns=NS, d=D, c=C, ct=CT, n_cores=N_CORES, act_w=1536, inplace=1,
          prefetch=3, use_ttr=0, use_quake=1, use_fastlog=1, rhs_sliced=1,
          split=1):
    # use_ttr=1 (InstTensorTensorReduce) crashes real HW (NRT INTERNAL)
    # even though CoreSim accepts it -- probed 2026-08-07; keep it off.
    ni = ns // 128                 # 4 n-tiles
    nk8 = d // 256                 # 2 DoubleRow K-steps
    nhb = (c + 4 * ct - 1) // (4 * ct)   # host 2048-wide row blocks
    # chunk descriptors (host_block, col_off, width), one per host block
    # (splitting the first block into single-bank chunks was tried and
    # regressed: tiny groups are consumer-limited, ~1.05us for 0.43us of
    # PE work)
    chunks = [(hb, 0, min(4 * ct, c - 4 * ct * hb)) for hb in range(nhb)]
    nch = len(chunks)

    nc = bacc.Bacc("TRN2", target_bir_lowering=False, debug=False,
                   num_devices=n_cores)
    x_nat = nc.dram_tensor("x_nat", [ns, d], bf16, kind="ExternalInput").ap()
    xtb_d = nc.dram_tensor("xtb", [d, ns], fp8, kind="ExternalInput").ap()
    wl = nc.dram_tensor("wl", [ns, d], bf16, kind="ExternalInput").ap()
    # W^T stored chunk-major ([nch*d, 4*ct], last chunk zero-padded) so
    # every DMA stride stays small (the flat [d, C] layout would need a
    # 100000-byte partition stride)
    wt = nc.dram_tensor("wt", [nhb * d, 4 * ct], fp8,
                        kind="ExternalInput").ap()
    out = nc.dram_tensor("out", [1, 1], f32, kind="ExternalOutput").ap()

    with tile.TileContext(nc) as tc:
        with (
            tc.tile_pool(name="persist", bufs=1) as pp,
            tc.tile_pool(name="stage", bufs=3) as sp,
            tc.tile_pool(name="wbuf",
                         bufs=(prefetch + 1) * (8 if rhs_sliced == 2 else 1)
                         ) as wbp,
            tc.tile_pool(name="scr", bufs=2) as scp,
        ):
            xtb = [pp.tile([128, 2, ns], fp8, tag=f"xtb{g}",
                           name=f"xtbs{g}") for g in range(nk8)]
            parts = pp.tile([128, ni * nch * 2], f32, tag="parts",
                            name="parts")
            ss = pp.tile([128, ni], f32, tag="ss", name="ss")
            tgt = pp.tile([128, ni], f32, tag="tgt", name="tgt")
            a_all = pp.tile([128, ni], f32, tag="a_all", name="a_all")
            a2_all = pp.tile([128, ni], f32, tag="a2_all", name="a2_all")
            ones = pp.tile([128, 1], f32, tag="ones", name="ones")

            # xT resident in SBUF -- gates the first matmuls
            for g in range(nk8):
                nc.sync.dma_start(
                    xtb[g][:],
                    xtb_d[g * 256:(g + 1) * 256, :].rearrange(
                        "(s p) n -> p s n", s=2))

            # W-chunk staging: one wide DMA per chunk, 4 DoubleRow k-pair
            # planes so rhs slices [:, 2g:2g+2, :] feed the matmuls
            def stage_chunk(ci, fine=False):
                hb, c0, cw = chunks[ci]
                rows = wt[hb * d:(hb + 1) * d, c0:c0 + cw]
                if rhs_sliced == 2:
                    # exact baseline staging: per-(g, jc) [128, 2, ct]
                    # tiles, rhs APs are whole tiles
                    wbt = {}
                    for jc in range((cw + ct - 1) // ct):
                        w0, w1 = jc * ct, min((jc + 1) * ct, cw)
                        for g in range(nk8):
                            wb = wbp.tile([128, 2, ct], fp8, tag="wbe",
                                          name="wbe")
                            nc.sync.dma_start(
                                wb[:, :, :w1 - w0],
                                rows[g * 256:(g + 1) * 256, w0:w1].rearrange(
                                    "(s p) c -> p s c", s=2))
                            wbt[(g, jc)] = wb
                    return wbt
                if not rhs_sliced:
                    # baseline-shaped staging: one [128, 2, cw] tile per
                    # DoubleRow k-group, rhs APs never slice the pair dim
                    wbg = []
                    for g in range(nk8):
                        wb = wbp.tile([128, 2, 4 * ct], fp8, tag=f"wbg{g}",
                                      name=f"wbg{g}")
                        nc.sync.dma_start(
                            wb[:, :, :cw],
                            rows[g * 256:(g + 1) * 256, :cw].rearrange(
                                "(s p) c -> p s c", s=2))
                        wbg.append(wb)
                    return wbg
                wb = wbp.tile([128, 4, 4 * ct], fp8, tag="wb", name="wb")
                if fine:   # per-bank DMAs so the first matmul starts ASAP
                    for jc in range((cw + ct - 1) // ct):
                        w0, w1 = jc * ct, min((jc + 1) * ct, cw)
                        nc.sync.dma_start(
                            wb[:, :, w0:w1],
                            rows[:, w0:w1].rearrange("(s p) c -> p s c", s=4))
                else:
                    nc.sync.dma_start(
                        wb[:, :, :cw],
                        rows[:, :cw].rearrange("(s p) c -> p s c", s=4))
                return wb

            # chunk0 first (gates the first matmuls), then the norm x
            # tiles (gate the first ACT at ~t+4us), then the deeper W
            # prefetch -- all squeezed into the same ~300GB/s DMA stream
            staged = {0: stage_chunk(0, fine=True)}

            # norms, DVE only: ss = sum(x^2)/S^2, a = rsqrt(ss), then the
            # Schraudolph pre-scale a2 = a * 2^23/ln2
            for i in range(ni):
                xa = sp.tile([128, d], bf16, tag="xa", name="xa")
                nc.sync.dma_start(xa[:], x_nat[i * 128:(i + 1) * 128, :])
                sq = scp.tile([128, d], f32, tag="sq", name="sq")
                if use_ttr:
                    nc.vector.tensor_tensor_reduce(
                        out=sq[:], in0=xa[:], in1=xa[:], scale=1.0 / (S * S),
                        scalar=0.0, op0=ALU.mult, op1=ALU.add,
                        accum_out=ss[:, i:i + 1])
                else:
                    nc.vector.tensor_mul(sq[:], xa[:], xa[:])
                    nc.vector.reduce_sum(ss[:, i:i + 1], sq[:], axis=AX.X)
            if not use_ttr:
                # fold the S factor: a = rsqrt(ss / S^2) = S / ||x||
                nc.vector.tensor_scalar_mul(ss[:], ss[:], 1.0 / (S * S))
            if use_quake:
                y0i = pp.tile([128, ni], i32, tag="y0i", name="y0i")
                yt = pp.tile([128, ni], f32, tag="yt", name="yt")
                rt = pp.tile([128, ni], f32, tag="rt", name="rt")
                # seed: bits(y0) = MAGIC - bits(ss)/2 (int arithmetic done
                # in f32; the low bits it rounds away are noise the Newton
                # steps absorb)
                nc.vector.tensor_scalar(out=y0i[:], in0=ss[:].bitcast(i32),
                                        scalar1=-0.5, scalar2=RSQ_MAGIC,
                                        op0=ALU.mult, op1=ALU.add)
                ycur = y0i[:].bitcast(f32)
                for it in range(2):
                    dst_y = a_all if it == 1 else yt
                    nc.vector.tensor_mul(rt[:], ycur, ycur)
                    nc.vector.tensor_mul(rt[:], rt[:], ss[:])
                    nc.vector.tensor_scalar(out=rt[:], in0=rt[:],
                                            scalar1=-0.5, scalar2=1.5,
                                            op0=ALU.mult, op1=ALU.add)
                    nc.vector.tensor_mul(dst_y[:], ycur, rt[:])
                    ycur = dst_y[:]
            else:
                ut = pp.tile([128, ni], f32, tag="ut", name="ut")
                nc.scalar.activation(ut[:], ss[:], AF.Sqrt)
                nc.vector.reciprocal(a_all[:], ut[:])
            nc.vector.tensor_scalar_mul(a2_all[:], a_all[:], EXP_A)
            nc.vector.memset(ones[:], 1.0)
            for ci in range(1, min(prefetch, nch)):
                staged[ci] = stage_chunk(ci)

            # target-logit work for n-tile i: one fused DVE mul+reduce
            def tgt_work(i):
                xa2 = sp.tile([128, d], bf16, tag="xa2", name="xa2")
                nc.sync.dma_start(xa2[:], x_nat[i * 128:(i + 1) * 128, :])
                wla = sp.tile([128, d], bf16, tag="wla", name="wla")
                nc.sync.dma_start(wla[:], wl[i * 128:(i + 1) * 128, :])
                pr = scp.tile([128, d], f32, tag="pr", name="pr")
                if use_ttr:
                    nc.vector.tensor_tensor_reduce(
                        out=pr[:], in0=xa2[:], in1=wla[:], scale=1.0,
                        scalar=0.0, op0=ALU.mult, op1=ALU.add,
                        accum_out=tgt[:, i:i + 1])
                else:
                    nc.vector.tensor_mul(pr[:], xa2[:], wla[:])
                    nc.vector.reduce_sum(tgt[:, i:i + 1], pr[:], axis=AX.X)

            # main loop: 49 chunks x 4 n-tiles. One 4-bank PSUM group per
            # (chunk, i); ScalarE consumes banks 0-2 (exact exp, in-place,
            # fused accum), VectorE consumes bank 3 (fast-exp + reduce).
            # two separate PSUM pools so the ACT (banks 0-2) and DVE
            # (bank 3) consumers are independent tiles -- a single 4-bank
            # tile made the scheduler serialize the DVE read behind the
            # ACT accumulator-read, stalling the PE ~1.1us every 2 groups
            tgt_done = set()
            with (
                tc.tile_pool(name="psumA", bufs=2, space="PSUM") as psa,
                tc.tile_pool(name="psumD", bufs=2, space="PSUM") as psd,
            ):
                for ci in range(nch):
                    wb = staged.pop(ci)
                    if ci + prefetch < nch:
                        staged[ci + prefetch] = stage_chunk(ci + prefetch)
                    cw = chunks[ci][2]
                    aw = min(act_w, cw)
                    njc = (cw + ct - 1) // ct
                    for i in range(ni):
                        ps = psa.tile([128, 3 * ct], f32, tag="ps",
                                      name="ps")
                        pd = psd.tile([128, ct], f32, tag="pd", name="pd")
                        for g in range(nk8):
                            lhs = xtb[g][:, :, i * 128:(i + 1) * 128]
                            for jc in range(njc):
                                w0, w1 = jc * ct, min((jc + 1) * ct, cw)
                                if rhs_sliced == 2:
                                    rhs = wb[(g, jc)][:, :, :w1 - w0]
                                elif rhs_sliced:
                                    rhs = wb[:, 2 * g:2 * g + 2, w0:w1]
                                else:
                                    rhs = wb[g][:, :, w0:w1]
                                dst = (ps[:, w0:w1] if jc < 3
                                       else pd[:, :w1 - w0])
                                nc.tensor.matmul(
                                    dst, lhs, rhs,
                                    start=(g == 0), stop=(g == nk8 - 1),
                                    perf_mode=(
                                        mybir.MatmulPerfMode.DoubleRow))
                        col = 2 * (i * nch + ci)
                        if inplace:
                            act_dst = ps[:, :aw]
                        else:
                            es = scp.tile([128, 3 * ct], bf16, tag="es",
                                          name="es")
                            act_dst = es[:, :aw]
                        nc.scalar.activation(
                            act_dst, ps[:, :aw], AF.Exp,
                            scale=a_all[:, i:i + 1],
                            accum_out=parts[:, col:col + 1])
                        dw = cw - aw
                        if dw > 0:
                            ti = scp.tile([128, ct], i32, tag="ti",
                                          name="ti")
                            nc.vector.tensor_scalar(
                                out=ti[:, :dw], in0=pd[:, :dw],
                                scalar1=a2_all[:, i:i + 1], scalar2=EXP_B,
                                op0=ALU.mult, op1=ALU.add)
                            nc.vector.reduce_sum(parts[:, col + 1:col + 2],
                                                 ti[:, :dw].bitcast(f32),
                                                 axis=AX.X)
                        else:
                            nc.vector.memset(parts[:, col + 1:col + 2], 0.0)
                    # spread the 4 tgt tiles across the loop interior
                    step = max(nch // (ni + 1), 1)
                    if ci % step == 0 and 1 <= ci // step <= ni \
                            and ci // step - 1 not in tgt_done:
                        tgt_work(ci // step - 1)
                        tgt_done.add(ci // step - 1)
                for i in range(ni):
                    if i not in tgt_done:
                        tgt_work(i)

            # epilogue: per-core partial = sum_i (log den_i - s*tgt_i)
            t1 = pp.tile([128, ni], f32, tag="t1", name="t1")
            e1 = pp.tile([128, ni], f32, tag="e1", name="e1")
            e2 = pp.tile([128, ni], f32, tag="e2", name="e2")
            loc = pp.tile([128, ni], f32, tag="loc", name="loc")
            den = pp.tile([128, ni], f32, tag="den", name="den")
            lg = pp.tile([128, ni], f32, tag="lg", name="lg")
            v = pp.tile([128, ni], f32, tag="v", name="v")
            rowv = pp.tile([128, 1], f32, tag="rowv", name="rowv")
            res = pp.tile([1, 1], f32, tag="res", name="res")

            for i in range(ni):
                nc.vector.reduce_sum(
                    loc[:, i:i + 1],
                    parts[:, 2 * i * nch:2 * (i + 1) * nch], axis=AX.X)
            nc.vector.tensor_mul(t1[:], a_all[:], tgt[:])   # s * tgt cosine
            nc.scalar.activation(e2[:], t1[:], AF.Exp)      # same table set
            nc.vector.tensor_scalar_mul(e1[:], e2[:], float(np.exp(-SM)))
            nc.vector.tensor_sub(e1[:], e1[:], e2[:])       # e^(t1-SM)-e^t1
            nc.vector.tensor_add(den[:], loc[:], e1[:])
            if use_fastlog:
                # fast-log: lg = (bits(den) - B) * ln2/2^23
                nc.vector.tensor_scalar(out=lg[:], in0=den[:].bitcast(i32),
                                        scalar1=LOG_K, scalar2=-LOG_B * LOG_K,
                                        op0=ALU.mult, op1=ALU.add)
            else:
                nc.scalar.activation(lg[:], den[:], AF.Ln)
            nc.vector.tensor_sub(v[:], lg[:], t1[:])
            nc.vector.reduce_sum(rowv[:], v[:], axis=AX.X)
            with tc.tile_pool(name="psum1", bufs=1, space="PSUM") as psp1:
                pss = psp1.tile([1, 1], f32, tag="pss", name="pss")
                nc.tensor.matmul(pss[:], rowv[:], ones[:], start=True,
                                 stop=True)
                nc.vector.tensor_scalar_mul(res[:], pss[:], 1.0)
            nc.sync.dma_start(out[:], res[:])

    nc.compile()
    return nc


def in_maps(x, W, labels, n_cores=N_CORES):
    ns = x.shape[0] // n_cores
    x = np.ascontiguousarray(np.asarray(x, dtype=np.float32))
    W = np.ascontiguousarray(np.asarray(W, dtype=np.float32))
    lab = np.asarray(labels).astype(np.int64)
    c, d = W.shape
    nch = (c + 2048 - 1) // 2048
    wtf = W.T.astype(np_fp8)                            # [D, C]
    wt = np.zeros((nch * d, 2048), np_fp8)              # chunk-major
    for ci in range(nch):
        cw = min(2048, c - ci * 2048)
        wt[ci * d:(ci + 1) * d, :cw] = wtf[:, ci * 2048:ci * 2048 + cw]
    wlg = np.ascontiguousarray(W[lab].astype(np_bf16))  # [N, D]
    maps = []
    for cid in range(n_cores):
        xs = x[cid * ns:(cid + 1) * ns]
        maps.append({
            "x_nat": np.ascontiguousarray(xs.astype(np_bf16)),
            "xtb": np.ascontiguousarray(xs.T.astype(np_fp8)),
            "wl": np.ascontiguousarray(wlg[cid * ns:(cid + 1) * ns]),
            "wt": wt,
        })
    return maps


def gather(results, n=N):
    """Host-side unshard: mean over the per-core partial sums + margin."""
    tot = sum(float(np.asarray(r["out"], dtype=np.float32).reshape(()))
              for r in results)
    return np.float32(tot / n + SM)


_CACHE = {}


def _get_nc():
    if "nc" not in _CACHE:
        _CACHE["nc"] = build(inplace=0, prefetch=14)
    return _CACHE["nc"]


def kernel(x, W, labels):
    nc = _get_nc()
    res = run_bass_kernel_spmd(nc, in_maps(x, W, labels),
                               core_ids=list(range(N_CORES)))
    return gather(res.results).reshape(())
